# revision 1
# baseline (speedup 1.0000x reference)
"""Bass/Tile attention kernel for TRN2 — per-core program builder.

Per-core work (SPMD, core c of 8): batch b = c//2, query-half = c%2.
Inputs (per core DRAM):
  xt  : [D, S]  bf16   X[b]^T with token columns permuted so cols 0..SQ-1
                       are this core's query tokens (K/V use all S tokens;
                       token order is irrelevant for softmax/PV).
  wq/wk/wv/wo : [D, D] bf16 (natural [d_in, d_out] / [d, e] layout)
  bq/bk/bv/bo : [D] fp32
Output:
  ot  : [D, SQ] fp32   O^T for this core's query half (host transposes).

Layouts on chip (P=128 partitions):
  xt_sb[p, c, t]  = X^T[c*128+p, t]                 (bf16)
  kt[pair][p, t]  = K^T[pair*128+p, t]              (bf16)  pair = 2 heads
  qt[pair][p, q]  = Q^T[pair*128+p, q]              (bf16)
  vt[pair][p, tk, h*65+j] = V[tk*128+p, pair*128+h*64+j] for j<64,
                            1.0 for j==64 (augmented ones col)   (bf16)
  zt[pair][p, q]  = Z^T[pair*128+p, q] (normalized) (bf16)

Attention per (pair, 512-query chunk qc), one PSUM accumulation chain per
bank (zero-region rule: a bank may hold only one chain at a time):
  for kc in range(S/128):
    scores^T[k,q] both heads -> PSUM tile [128, 2, 512] (2 banks), via
      row-packed single-shot matmuls (contraction d=64: head-lo array rows
      0-63, head-hi rows 64-127).
    probs = exp(0.125*scores): one ACT instruction [128, 1024] -> bf16 SBUF.
    PV: per head, stationary [V_h | 1] (M=65): z_h PSUM [65, 512] chain
      accumulated over kc; row 64 = softmax denominator (rowsum).
  reciprocal of row 64, DRAM-bounce partition-broadcast to [64, 512],
  multiply rows 0-63 -> zt (head-hi goes via a bf16 stage + partition-shift
  DMA into zt[pair][64:128]).
O-proj: O^T[e,q] = sum_d Wo[d,e] Z^T[d,q] + bo, accumulated over 8
pair-chunks of d, evicted with bias add, DMA to ot.
"""

import numpy as np
import ml_dtypes
from contextlib import ExitStack

import concourse.bass as bass
import concourse.tile as tile
from concourse import bacc, mybir

F32 = mybir.dt.float32
BF16 = mybir.dt.bfloat16
P = 128


def build_attention_nc(S_full=2048, SQ=1024, D=1024, H=16):
    DK = D // H
    assert DK == 64
    NPAIR = D // P            # head pairs (128 dims each)
    NQUAD = NPAIR // 2
    KC = S_full // P          # k chunks
    TOKC = S_full // 512      # 512-token col chunks for K proj
    NTOK = S_full // P        # 128-token chunks for V proj
    QCN = SQ // 512           # 512-query chunks
    DIN = D // P              # input-dim chunks

    nc = bacc.Bacc("TRN2", target_bir_lowering=False, debug=False)

    xt_d = nc.dram_tensor("xt", [D, S_full], BF16, kind="ExternalInput").ap()
    wq_d = nc.dram_tensor("wq", [D, D], BF16, kind="ExternalInput").ap()
    wk_d = nc.dram_tensor("wk", [D, D], BF16, kind="ExternalInput").ap()
    wv_d = nc.dram_tensor("wv", [D, D], BF16, kind="ExternalInput").ap()
    wo_d = nc.dram_tensor("wo", [D, D], BF16, kind="ExternalInput").ap()
    bq_d = nc.dram_tensor("bq", [D], F32, kind="ExternalInput").ap()
    bk_d = nc.dram_tensor("bk", [D], F32, kind="ExternalInput").ap()
    bv_d = nc.dram_tensor("bv", [D], F32, kind="ExternalInput").ap()
    bo_d = nc.dram_tensor("bo", [D], F32, kind="ExternalInput").ap()
    ot_d = nc.dram_tensor("ot", [D, SQ], F32, kind="ExternalOutput").ap()

    xt_r = xt_d.rearrange("(c p) t -> p c t", p=P)
    wq_r = wq_d.rearrange("(c p) n -> p c n", p=P)
    wk_r = wk_d.rearrange("(c p) n -> p c n", p=P)
    wv_r = wv_d.rearrange("(c p) n -> p c n", p=P)
    wo_r = wo_d.rearrange("(c p) n -> p c n", p=P)
    bq_r = bq_d.rearrange("(c p) -> p c", p=P)
    bk_r = bk_d.rearrange("(c p) -> p c", p=P)
    bo_r = bo_d.rearrange("(c p) -> p c", p=P)
    bv_r = bv_d.rearrange("(a d) -> a d", a=1)

    EXP = mybir.ActivationFunctionType.Exp

    with tile.TileContext(nc) as tc, ExitStack() as ctx:
        const = ctx.enter_context(tc.tile_pool(name="const", bufs=1))
        big = ctx.enter_context(tc.tile_pool(name="big", bufs=1))
        wpool = ctx.enter_context(tc.tile_pool(name="wpool", bufs=2))
        work = ctx.enter_context(tc.tile_pool(name="work", bufs=3))
        probs_pool = ctx.enter_context(tc.tile_pool(name="probs", bufs=6))
        # PSUM budget (8 banks): scores 2x2 + z 3 + proj 1
        spsum = ctx.enter_context(tc.tile_pool(name="spsum", bufs=2, space="PSUM"))
        zpsum = ctx.enter_context(tc.tile_pool(name="zpsum", bufs=3, space="PSUM"))
        ppsum = ctx.enter_context(tc.tile_pool(name="ppsum", bufs=1, space="PSUM"))
        dramp = ctx.enter_context(tc.tile_pool(name="dramp", bufs=2, space="DRAM"))

        # ---- constants ----
        bq_t = const.tile([P, DIN], F32)
        nc.sync.dma_start(bq_t[:], bq_r[:, :])
        bk_t = const.tile([P, DIN], F32)
        nc.sync.dma_start(bk_t[:], bk_r[:, :])
        bo_t = const.tile([P, DIN], F32)
        nc.sync.dma_start(bo_t[:], bo_r[:, :])
        bvb = const.tile([P, D], F32)
        nc.sync.dma_start(bvb[:], bv_r[0:1, :].to_broadcast((P, D)))

        # ---- X^T + Wk loads, chunk-interleaved so K-proj starts early ----
        xt_t = big.tile([P, DIN, S_full], BF16, tag="xt")
        wk_t = wpool.tile([P, DIN, D], BF16, tag="w", name="wk_t")
        for c in range(DIN):
            nc.sync.dma_start(wk_t[:, c, :], wk_r[:, c, :])
            nc.sync.dma_start(xt_t[:, c, :], xt_r[:, c, :])

        kt = [big.tile([P, S_full], BF16, tag=f"kt{p}", name=f"kt{p}")
              for p in range(NPAIR)]
        qt = [big.tile([P, SQ], BF16, tag=f"qt{p}", name=f"qt{p}")
              for p in range(NPAIR)]
        # per-pair augmented V: 2 heads x (64 data + 1 ones col)
        vt = [big.tile([P, NTOK, 130], BF16, tag=f"vt{p}", name=f"vt{p}")
              for p in range(NPAIR)]
        zt = [big.tile([P, SQ], BF16, tag=f"zt{p}", name=f"zt{p}")
              for p in range(NPAIR)]

        # ones columns of vt (col 64 of each head's 65-wide block)
        for pr in range(NPAIR):
            ones_ap = vt[pr].rearrange("p t (h c) -> p t h c", c=65)[:, :, :, 64:65]
            nc.vector.memset(ones_ap, 1.0)

        # ---- weight loads (double-buffered slots) ----
        def load_w(w_r, nm):
            w_t = wpool.tile([P, DIN, D], BF16, tag="w", name=nm)
            for c in range(DIN):
                nc.sync.dma_start(w_t[:, c, :], w_r[:, c, :])
            return w_t

        wv_t = load_w(wv_r, "wv_t")

        # ---- K^T projection for all pairs (frees wk's slot for wq) ----
        for pr in range(NPAIR):
            for t in range(TOKC):
                ps = spsum.tile([P, 512], F32, tag="sc", name="psk")
                for c in range(DIN):
                    nc.tensor.matmul(
                        ps[:],
                        wk_t[:, c, pr * P:(pr + 1) * P],
                        xt_t[:, c, t * 512:(t + 1) * 512],
                        start=(c == 0), stop=(c == DIN - 1),
                    )
                nc.vector.tensor_scalar_add(
                    kt[pr][:, t * 512:(t + 1) * 512], ps[:], bk_t[:, pr:pr + 1]
                )
        wq_t = load_w(wq_r, "wq_t")

        # ---- V/Q projections + attention, per quad / pair ----
        for g in range(NQUAD):
            prs = (2 * g, 2 * g + 1)
            # V projection for the quad's 256 columns; evict 128 cols
            # into each pair tile (strided dest skips ones columns)
            for t in range(NTOK):
                ps = ppsum.tile([P, 256], F32, tag="proj", name="psv")
                for c in range(DIN):
                    nc.tensor.matmul(
                        ps[:],
                        xt_t[:, c, t * P:(t + 1) * P],
                        wv_t[:, c, g * 256:(g + 1) * 256],
                        start=(c == 0), stop=(c == DIN - 1),
                    )
                for u in range(2):
                    dst = vt[prs[u]].rearrange(
                        "p t (h c) -> p t h c", c=65)[:, t, :, 0:64]
                    src = ps[:, u * 128:(u + 1) * 128].rearrange(
                        "p (h c) -> p h c", c=64)
                    bsrc = bvb[:, (g * 256 + u * 128):
                               (g * 256 + (u + 1) * 128)
                               ].rearrange("p (h c) -> p h c", c=64)
                    nc.vector.tensor_add(dst, src, bsrc)
            # Q^T projection for the quad's two pairs (all q chunks)
            for pr in prs:
                for t in range(QCN):
                    ps = ppsum.tile([P, 512], F32, tag="proj",
                                    name="psq")
                    for c in range(DIN):
                        nc.tensor.matmul(
                            ps[:],
                            wq_t[:, c, pr * P:(pr + 1) * P],
                            xt_t[:, c, t * 512:(t + 1) * 512],
                            start=(c == 0), stop=(c == DIN - 1),
                        )
                    nc.vector.tensor_scalar_add(
                        qt[pr][:, t * 512:(t + 1) * 512], ps[:],
                        bq_t[:, pr:pr + 1]
                    )

            # ---- attention for each pair of the quad ----
            for pr in prs:
                vpr = vt[pr].rearrange("p t (h c) -> p t h c", c=65)
                for qc in range(QCN):
                    qsl = slice(qc * 512, (qc + 1) * 512)
                    za = zpsum.tile([P, 512], F32, tag="z", name=f"za{pr}_{qc}")
                    zb = zpsum.tile([P, 512], F32, tag="z", name=f"zb{pr}_{qc}")
                    for kc in range(KC):
                        sq = spsum.tile([P, 2, 512], F32, tag="sc", name="sq")
                        nc.tensor.matmul(
                            sq[:, 0, :],
                            kt[pr][0:64, kc * P:(kc + 1) * P],
                            qt[pr][0:64, qsl],
                            start=True, stop=True,
                        )
                        nc.tensor.matmul(
                            sq[:, 1, :],
                            kt[pr][64:128, kc * P:(kc + 1) * P],
                            qt[pr][64:128, qsl],
                            start=True, stop=True,
                        )
                        pq = probs_pool.tile([P, 2, 512], BF16, tag="probs",
                                             name="pq")
                        nc.scalar.activation(pq[:], sq[:], EXP, scale=0.125)
                        nc.tensor.matmul(
                            za[0:65, :], vpr[:, kc, 0, :], pq[:, 0, :],
                            start=(kc == 0), stop=(kc == KC - 1),
                        )
                        nc.tensor.matmul(
                            zb[0:65, :], vpr[:, kc, 1, :], pq[:, 1, :],
                            start=(kc == 0), stop=(kc == KC - 1),
                        )
                    # normalize + evict Z^T; 1/rowsum partition-broadcast
                    # goes through a DRAM bounce (SBUF->SBUF bcast illegal)
                    rcin = work.tile([P, 2, 512], F32, tag="rcin", name="rcin", bufs=2)
                    rsc = dramp.tile([2, 512], F32, tag="rsc", name="rsc")
                    # Stage rowsum rows to SBUF via ScalarE (the approx-recip
                    # custom-DVE op misreads PSUM on HW), DRAM-bounce them to
                    # a partition-broadcast tile, then take the reciprocal
                    # there (the custom op also requires base partition 0).
                    nc.scalar.copy(rcin[64:65, 0, :], za[64:65, :])
                    nc.vector.tensor_copy(rcin[64:65, 1, :], zb[64:65, :])
                    nc.sync.dma_start(rsc[0:1, :], rcin[64:65, 0, :])
                    nc.sync.dma_start(rsc[1:2, :], rcin[64:65, 1, :])
                    rbr = work.tile([P, 2, 512], F32, tag="rbc", name="rbr")
                    nc.sync.dma_start(rbr[0:64, 0, :],
                                      rsc[0:1, :].to_broadcast((64, 512)))
                    nc.sync.dma_start(rbr[0:64, 1, :],
                                      rsc[1:2, :].to_broadcast((64, 512)))
                    rb = work.tile([P, 2, 512], F32, tag="rbc", name="rb")
                    nc.vector.reciprocal_approx_fast(rb[0:64, :, :],
                                                     rbr[0:64, :, :])
                    nc.vector.tensor_mul(zt[pr][0:64, qsl], za[0:64, :],
                                         rb[0:64, 0, :])
                    zs = work.tile([P, 512], BF16, tag="zstage", name="zs")
                    nc.vector.tensor_mul(zs[0:64, :], zb[0:64, :],
                                         rb[0:64, 1, :])
                    nc.sync.dma_start(zt[pr][64:128, qsl], zs[0:64, :])

        # ---- output projection ----
        wo_t = load_w(wo_r, "wo_t")
        for qc in range(QCN):
            oqsl = slice(qc * 512, (qc + 1) * 512)
            for ec in range(DIN):
                ps = spsum.tile([P, 512], F32, tag="sc", name="pso")
                for dc in range(NPAIR):
                    nc.tensor.matmul(
                        ps[:],
                        wo_t[:, dc, ec * P:(ec + 1) * P],
                        zt[dc][:, oqsl],
                        start=(dc == 0), stop=(dc == NPAIR - 1),
                    )
                st = work.tile([P, 512], F32, tag="stage", name="st", bufs=2)
                nc.vector.tensor_scalar_add(st[:], ps[:], bo_t[:, ec:ec + 1])
                nc.sync.dma_start(ot_d[ec * P:(ec + 1) * P, oqsl], st[:])

    nc.compile()
    return nc



# ---------------- host-side entry point ----------------

BF = ml_dtypes.bfloat16
_B, _S, _D, _H = 4, 2048, 1024, 16
_SQ = _S // 2
_NC_CACHE = None


def _get_nc():
    global _NC_CACHE
    if _NC_CACHE is None:
        _NC_CACHE = build_attention_nc(S_full=_S, SQ=_SQ, D=_D, H=_H)
    return _NC_CACHE


def kernel(X, Wq, bq, Wk, bk, Wv, bv, Wo, bo):
    """Full-input multi-head attention on 8 TRN2 NeuronCores.

    Sharding: core c handles batch c//2, query-half c%2 (no collectives;
    K/V are recomputed per query-half). Inputs are cast to bf16 on host
    (matmul precision), X is transposed per core with its query half
    leading; output is fp32 [B, S, D].
    """
    from concourse.bass_utils import run_bass_kernel_spmd

    X = np.asarray(X, dtype=np.float32)
    bq = np.asarray(bq, dtype=np.float32)
    bk = np.asarray(bk, dtype=np.float32)
    bv = np.asarray(bv, dtype=np.float32)
    bo = np.asarray(bo, dtype=np.float32)
    wqb = np.ascontiguousarray(np.asarray(Wq, dtype=np.float32).astype(BF))
    wkb = np.ascontiguousarray(np.asarray(Wk, dtype=np.float32).astype(BF))
    wvb = np.ascontiguousarray(np.asarray(Wv, dtype=np.float32).astype(BF))
    wob = np.ascontiguousarray(np.asarray(Wo, dtype=np.float32).astype(BF))

    in_maps = []
    for c in range(8):
        b, half = c // 2, c % 2
        order = np.concatenate([
            np.arange(half * _SQ, (half + 1) * _SQ),
            np.arange((1 - half) * _SQ, (2 - half) * _SQ),
        ])
        xtp = np.ascontiguousarray(X[b][order, :].T.astype(BF))
        in_maps.append({
            "xt": xtp, "wq": wqb, "wk": wkb, "wv": wvb, "wo": wob,
            "bq": bq, "bk": bk, "bv": bv, "bo": bo,
        })

    nc = _get_nc()
    res = run_bass_kernel_spmd(nc, in_maps, list(range(8)))

    out = np.empty((_B, _S, _D), np.float32)
    for c in range(8):
        b, half = c // 2, c % 2
        out[b, half * _SQ:(half + 1) * _SQ, :] = res.results[c]["ot"].T
    return out



# revision 5
# speedup vs baseline: 1.2136x; 1.2136x over previous
"""Bass/Tile attention kernel for TRN2 — per-core program builder (v2).

Sharding (SPMD, core c of 8): batch b = c//2, head-half hh = c%2.
Each core computes Q/K/V projections for its 8 heads only (512 of the
1024 model dims), full attention for those heads over all 2048 tokens,
and a PARTIAL output projection O_part = Z_local @ Wo[local_rows].
The host sums the two partials per batch and adds bo (no collective).

Per-core DRAM inputs:
  xt  : [D, S]   bf16  X[b]^T (full model dims, all tokens)
  wq/wk/wv : [D, DL]  bf16  column-slice for this head-half
  wo  : [DL, D]  bf16  row-slice for this head-half
  bq/bk/bv : [DL] fp32
Output:
  ot  : [D, S]  fp32  partial O^T for this batch (host: sum pair + ^T + bo)

On-chip layouts (P=128 partitions):
  xt_sb[p, c, t] = X^T[c*128+p, t]                  bf16
  kt[pr][p, t]   = K^T[pr*128+p, t]   (pr = local pair of heads)
  qt2[qb][p, pr, q] = Q^T[pr*128+p, qc*512+q]  (double-buffered per qc)
  vt[pr][p, tk, h*65+j] = V[tk*128+p, pr*128+h*64+j] for j<64,
                          1.0 for j==64 (ones col -> softmax denom)
  zt[pr][p, q]   = normalized Z^T

Attention inner loop per (qc, pr): 16 kc chunks; per kc two row-packed
score matmuls (contraction 64 per head, PE rows 0-63 / 64-127) into a
2-bank PSUM tile, one ACT exp -> bf16 probs, and two M=65 PV matmuls
accumulating za/zb (row 64 = denominator).  PV is emitted LAG=2 kc
behind scores so the PE stream never blocks on the scalar engine.
Q-proj for the next qc and O-proj for the previous qc are emitted as
filler chains inside the kc stream (separate 1-bank PSUM pool).
"""

import numpy as np
import ml_dtypes
from contextlib import ExitStack

import concourse.bass as bass
import concourse.tile as tile
from concourse import bacc, mybir

F32 = mybir.dt.float32
BF16 = mybir.dt.bfloat16
P = 128


def build_attention_nc(S=2048, D=1024, DL=512):
    NPAIR = DL // P           # 4 local head pairs
    DIN = D // P              # 8 input-dim chunks
    KC = S // P               # 16 key chunks
    QCN = S // 512            # 4 query chunks
    NTOK = S // P             # 16 token chunks (V proj)
    LAG = 2                   # PV lags scores by LAG kc steps

    nc = bacc.Bacc("TRN2", target_bir_lowering=False, debug=False)

    xt_d = nc.dram_tensor("xt", [D, S], BF16, kind="ExternalInput").ap()
    wq_d = nc.dram_tensor("wq", [D, DL], BF16, kind="ExternalInput").ap()
    wk_d = nc.dram_tensor("wk", [D, DL], BF16, kind="ExternalInput").ap()
    wv_d = nc.dram_tensor("wv", [D, DL], BF16, kind="ExternalInput").ap()
    wo_d = nc.dram_tensor("wo", [DL, D], BF16, kind="ExternalInput").ap()
    bq_d = nc.dram_tensor("bq", [DL], F32, kind="ExternalInput").ap()
    bk_d = nc.dram_tensor("bk", [DL], F32, kind="ExternalInput").ap()
    bv_d = nc.dram_tensor("bv", [DL], F32, kind="ExternalInput").ap()
    ot_d = nc.dram_tensor("ot", [D, S], F32, kind="ExternalOutput").ap()

    xt_r = xt_d.rearrange("(c p) t -> p c t", p=P)
    wq_r = wq_d.rearrange("(c p) n -> p c n", p=P)
    wk_r = wk_d.rearrange("(c p) n -> p c n", p=P)
    wv_r = wv_d.rearrange("(c p) n -> p c n", p=P)
    wo_r = wo_d.rearrange("(c p) n -> p c n", p=P)
    bq_r = bq_d.rearrange("(c p) -> p c", p=P)
    bk_r = bk_d.rearrange("(c p) -> p c", p=P)
    bv_r = bv_d.rearrange("(a d) -> a d", a=1)

    EXP = mybir.ActivationFunctionType.Exp

    with tile.TileContext(nc) as tc, ExitStack() as ctx:
        const = ctx.enter_context(tc.tile_pool(name="const", bufs=1))
        big = ctx.enter_context(tc.tile_pool(name="big", bufs=1))
        wpool = ctx.enter_context(tc.tile_pool(name="wpool", bufs=3))
        qpool = ctx.enter_context(tc.tile_pool(name="qpool", bufs=2))
        work = ctx.enter_context(tc.tile_pool(name="work", bufs=3))
        probs_pool = ctx.enter_context(tc.tile_pool(name="probs", bufs=5))
        # PSUM budget (8 banks): scores 2x2 + z 3 + proj-filler 1
        spsum = ctx.enter_context(tc.tile_pool(name="spsum", bufs=2, space="PSUM"))
        zpsum = ctx.enter_context(tc.tile_pool(name="zpsum", bufs=3, space="PSUM"))
        ppsum = ctx.enter_context(tc.tile_pool(name="ppsum", bufs=1, space="PSUM"))
        dramp = ctx.enter_context(tc.tile_pool(name="dramp", bufs=2, space="DRAM"))

        # ---- constants ----
        bq_t = const.tile([P, NPAIR], F32)
        nc.sync.dma_start(bq_t[:], bq_r[:, :])
        bk_t = const.tile([P, NPAIR], F32)
        nc.sync.dma_start(bk_t[:], bk_r[:, :])
        bvb = const.tile([P, DL], F32)
        nc.sync.dma_start(bvb[:], bv_r[0:1, :].to_broadcast((P, DL)))

        # ---- X^T + Wk loads, chunk-interleaved so K-proj starts early ----
        xt_t = big.tile([P, DIN, S], BF16, tag="xt")
        wk_t = wpool.tile([P, DIN, DL], BF16, tag="w", name="wk_t")
        for c in range(DIN):
            nc.sync.dma_start(wk_t[:, c, :], wk_r[:, c, :])
            nc.sync.dma_start(xt_t[:, c, :], xt_r[:, c, :])

        kt = [big.tile([P, S], BF16, tag=f"kt{p}", name=f"kt{p}")
              for p in range(NPAIR)]
        vt = [big.tile([P, NTOK, 130], BF16, tag=f"vt{p}", name=f"vt{p}")
              for p in range(NPAIR)]
        zt = [big.tile([P, S], BF16, tag=f"zt{p}", name=f"zt{p}")
              for p in range(NPAIR)]

        # ones columns of vt (col 64 of each head's 65-wide block)
        for pr in range(NPAIR):
            ones_ap = vt[pr].rearrange("p t (h c) -> p t h c", c=65)[:, :, :, 64:65]
            nc.vector.memset(ones_ap, 1.0)

        def load_w(w_r, nm, depth=DIN, width=DL):
            w_t = wpool.tile([P, depth, width], BF16, tag="w", name=nm)
            for c in range(depth):
                nc.sync.dma_start(w_t[:, c, :], w_r[:, c, :])
            return w_t

        wv_t = load_w(wv_r, "wv_t")

        # ---- K^T projection, all pairs (startup: uses spsum dbuf) ----
        for pr in range(NPAIR):
            for t in range(QCN):
                ps = spsum.tile([P, 2, 512], F32, tag="sc", name="psk")
                for c in range(DIN):
                    nc.tensor.matmul(
                        ps[:, 0, :],
                        wk_t[:, c, pr * P:(pr + 1) * P],
                        xt_t[:, c, t * 512:(t + 1) * 512],
                        start=(c == 0), stop=(c == DIN - 1),
                    )
                nc.vector.tensor_scalar_add(
                    kt[pr][:, t * 512:(t + 1) * 512], ps[:, 0, :],
                    bk_t[:, pr:pr + 1])

        wq_t = load_w(wq_r, "wq_t")

        # ---- V projection, all token chunks ----
        for t in range(NTOK):
            ps = spsum.tile([P, 2, 512], F32, tag="sc", name="psv")
            for c in range(DIN):
                nc.tensor.matmul(
                    ps[:, 0, :],
                    xt_t[:, c, t * P:(t + 1) * P],
                    wv_t[:, c, :],
                    start=(c == 0), stop=(c == DIN - 1),
                )
            for pr in range(NPAIR):
                dst = vt[pr].rearrange(
                    "p t (h c) -> p t h c", c=65)[:, t, :, 0:64]
                src = ps[:, 0, pr * P:(pr + 1) * P].rearrange(
                    "p (h c) -> p h c", c=64)
                bsrc = bvb[:, pr * P:(pr + 1) * P].rearrange(
                    "p (h c) -> p h c", c=64)
                nc.vector.tensor_add(dst, src, bsrc)

        # ---- Q^T projection helper (qc granularity, double-buffered) ----
        def qproj(qc, pool_):
            qt = qpool.tile([P, NPAIR, 512], BF16, tag="qt", name=f"qt{qc % 2}")
            for pr in range(NPAIR):
                ps = pool_.tile([P, 2, 512] if pool_ is spsum else [P, 512],
                                F32, tag="sc" if pool_ is spsum else "proj",
                                name="psq")
                pview = ps[:, 0, :] if pool_ is spsum else ps[:]
                for c in range(DIN):
                    nc.tensor.matmul(
                        pview,
                        wq_t[:, c, pr * P:(pr + 1) * P],
                        xt_t[:, c, qc * 512:(qc + 1) * 512],
                        start=(c == 0), stop=(c == DIN - 1),
                    )
                nc.vector.tensor_scalar_add(
                    qt[:, pr, :], pview, bq_t[:, pr:pr + 1])
            return qt

        qt_cur = qproj(0, spsum)
        wo_t = load_w(wo_r, "wo_t", depth=NPAIR, width=D)

        # ---- filler chain generators (run inside the kc stream) ----
        def oproj_chain(qc, ec):
            ps = ppsum.tile([P, 512], F32, tag="proj", name="pso")
            for dc in range(NPAIR):
                nc.tensor.matmul(
                    ps[:],
                    wo_t[:, dc, ec * P:(ec + 1) * P],
                    zt[dc][:, qc * 512:(qc + 1) * 512],
                    start=(dc == 0), stop=(dc == NPAIR - 1),
                )
            st = work.tile([P, 512], F32, tag="stage", name="st", bufs=2)
            nc.vector.tensor_copy(st[:], ps[:])
            nc.sync.dma_start(
                ot_d[ec * P:(ec + 1) * P, qc * 512:(qc + 1) * 512], st[:])

        def qproj_chain(qc, pr, qt):
            ps = ppsum.tile([P, 512], F32, tag="proj", name="psq")
            for c in range(DIN):
                nc.tensor.matmul(
                    ps[:],
                    wq_t[:, c, pr * P:(pr + 1) * P],
                    xt_t[:, c, qc * 512:(qc + 1) * 512],
                    start=(c == 0), stop=(c == DIN - 1),
                )
            nc.vector.tensor_scalar_add(qt[:, pr, :], ps[:], bq_t[:, pr:pr + 1])

        # ---- normalize + evict Z^T for one (pr, qc) ----
        # za/zb are staged to SBUF immediately (split across VectorE and
        # ScalarE) so their PSUM banks free up before the next pair's PV
        # needs them; the DMA-bounce broadcast and the normalize muls then
        # run entirely SBUF-side off the PE critical path.
        def normalize(pr, qc, za, zb):
            qsl = slice(qc * 512, (qc + 1) * 512)
            zsa = work.tile([P, 2, 512], F32, tag="zsa", name="zsa", bufs=2)
            nc.vector.tensor_copy(zsa[0:65, 0, :], za[0:65, :])
            nc.scalar.copy(zsa[0:65, 1, :], zb[0:65, :])
            rsc = dramp.tile([2, 512], F32, tag="rsc", name="rsc")
            nc.sync.dma_start(rsc[0:1, :], zsa[64:65, 0, :])
            nc.sync.dma_start(rsc[1:2, :], zsa[64:65, 1, :])
            rbr = work.tile([P, 2, 512], F32, tag="rbc", name="rbr")
            nc.sync.dma_start(rbr[0:64, 0, :],
                              rsc[0:1, :].to_broadcast((64, 512)))
            nc.sync.dma_start(rbr[0:64, 1, :],
                              rsc[1:2, :].to_broadcast((64, 512)))
            rb = work.tile([P, 2, 512], F32, tag="rbc", name="rb")
            nc.vector.reciprocal_approx_fast(rb[0:64, :, :], rbr[0:64, :, :])
            nc.vector.tensor_mul(zt[pr][0:64, qsl], zsa[0:64, 0, :],
                                 rb[0:64, 0, :])
            zs = work.tile([P, 512], BF16, tag="zstage", name="zs")
            nc.vector.tensor_mul(zs[0:64, :], zsa[0:64, 1, :],
                                 rb[0:64, 1, :])
            nc.sync.dma_start(zt[pr][64:128, qsl], zs[0:64, :])

        # ---- attention steady loop ----
        pv_q = []          # pending PV work: (pq, pr, qc, kc, za, zb)
        zacc = {}          # (pr, qc) -> (za, zb)
        fillers = []       # callables emitting one PE chain each

        def emit_pv(item):
            pq, pr, qc, kc, za, zb = item
            vpr = vt[pr].rearrange("p t (h c) -> p t h c", c=65)
            nc.tensor.matmul(
                za[0:65, :], vpr[:, kc, 0, :], pq[:, 0, :],
                start=(kc == 0), stop=(kc == KC - 1),
            )
            nc.tensor.matmul(
                zb[0:65, :], vpr[:, kc, 1, :], pq[:, 1, :],
                start=(kc == 0), stop=(kc == KC - 1),
            )
            if kc == KC - 1:
                normalize(pr, qc, za, zb)

        FILL_SLOTS = (3, 7, 11)   # kc positions where one filler chain runs

        for qc in range(QCN):
            qt_use = qt_cur
            # schedule fillers for this qc: Q-proj(qc+1) during pr3,
            # O-proj(qc-1) spread over pr0..pr2
            qnext = [None]
            if qc + 1 < QCN:
                qt_next = qpool.tile([P, NPAIR, 512], BF16, tag="qt",
                                     name=f"qt{(qc + 1) % 2}")
                qnext[0] = qt_next
            for pr in range(NPAIR):
                qsl = slice(qc * 512, (qc + 1) * 512)
                za = zpsum.tile([P, 512], F32, tag="z", name=f"za{pr}_{qc}")
                zb = zpsum.tile([P, 512], F32, tag="z", name=f"zb{pr}_{qc}")
                zacc[(pr, qc)] = (za, zb)
                for kc in range(KC):
                    sq = spsum.tile([P, 2, 512], F32, tag="sc", name="sq")
                    nc.tensor.matmul(
                        sq[:, 0, :],
                        kt[pr][0:64, kc * P:(kc + 1) * P],
                        qt_use[0:64, pr, :],
                        start=True, stop=True,
                    )
                    nc.tensor.matmul(
                        sq[:, 1, :],
                        kt[pr][64:128, kc * P:(kc + 1) * P],
                        qt_use[64:128, pr, :],
                        start=True, stop=True,
                    )
                    pq = probs_pool.tile([P, 2, 512], BF16, tag="pq",
                                         name="pq")
                    nc.scalar.activation(pq[:], sq[:], EXP, scale=0.125)
                    pv_q.append((pq, pr, qc, kc, za, zb))
                    if len(pv_q) > LAG:
                        emit_pv(pv_q.pop(0))
                    if kc in FILL_SLOTS and fillers:
                        fillers.pop(0)()
                # queue fillers now that this pair's scores are done
                if pr == 2 and qnext[0] is not None:
                    qt_next = qnext[0]
                    for fpr in range(NPAIR):
                        fillers.append(
                            lambda q=qc + 1, p=fpr, t=qt_next:
                            qproj_chain(q, p, t))
                if pr == 0 and qc > 0:
                    for ec in range(DIN):
                        fillers.append(
                            lambda q=qc - 1, e=ec: oproj_chain(q, e))
            if qnext[0] is not None:
                qt_cur = qnext[0]

        # drain PV queue, remaining fillers, and final O-proj
        while pv_q:
            emit_pv(pv_q.pop(0))
        while fillers:
            fillers.pop(0)()
        for ec in range(DIN):
            oproj_chain(QCN - 1, ec)

    nc.compile()
    return nc


# ---------------- host-side entry point ----------------

BF = ml_dtypes.bfloat16
_B, _S, _D, _H = 4, 2048, 1024, 16
_DL = _D // 2
_NC_CACHE = None


def _get_nc():
    global _NC_CACHE
    if _NC_CACHE is None:
        _NC_CACHE = build_attention_nc(S=_S, D=_D, DL=_DL)
    return _NC_CACHE


def kernel(X, Wq, bq, Wk, bk, Wv, bv, Wo, bo):
    """Full-input multi-head attention on 8 TRN2 NeuronCores.

    Sharding: core c handles batch c//2 and head-half c%2 (8 heads).
    Each core returns a partial O^T (its heads' contribution); the host
    sums the two partials per batch and adds bo. No collectives.
    """
    from concourse.bass_utils import run_bass_kernel_spmd

    X = np.asarray(X, dtype=np.float32)
    bo = np.asarray(bo, dtype=np.float32)
    wq_f = np.asarray(Wq, dtype=np.float32)
    wk_f = np.asarray(Wk, dtype=np.float32)
    wv_f = np.asarray(Wv, dtype=np.float32)
    wo_f = np.asarray(Wo, dtype=np.float32)
    bq_f = np.asarray(bq, dtype=np.float32)
    bk_f = np.asarray(bk, dtype=np.float32)
    bv_f = np.asarray(bv, dtype=np.float32)

    xts = [np.ascontiguousarray(X[b].T.astype(BF)) for b in range(_B)]
    in_maps = []
    for c in range(8):
        b, hh = c // 2, c % 2
        dsl = slice(hh * _DL, (hh + 1) * _DL)
        in_maps.append({
            "xt": xts[b],
            "wq": np.ascontiguousarray(wq_f[:, dsl].astype(BF)),
            "wk": np.ascontiguousarray(wk_f[:, dsl].astype(BF)),
            "wv": np.ascontiguousarray(wv_f[:, dsl].astype(BF)),
            "wo": np.ascontiguousarray(wo_f[dsl, :].astype(BF)),
            "bq": np.ascontiguousarray(bq_f[dsl]),
            "bk": np.ascontiguousarray(bk_f[dsl]),
            "bv": np.ascontiguousarray(bv_f[dsl]),
        })

    nc = _get_nc()
    res = run_bass_kernel_spmd(nc, in_maps, list(range(8)))

    out = np.empty((_B, _S, _D), np.float32)
    for b in range(_B):
        acc = res.results[2 * b]["ot"] + res.results[2 * b + 1]["ot"]
        out[b] = acc.T + bo[None, :]
    return out


# revision 12
# speedup vs baseline: 1.4929x; 1.2302x over previous
"""Bass/Tile attention kernel for TRN2 — per-core program builder (v2).

Sharding (SPMD, core c of 8): batch b = c//2, head-half hh = c%2.
Each core computes Q/K/V projections for its 8 heads only (512 of the
1024 model dims), full attention for those heads over all 2048 tokens,
and a PARTIAL output projection O_part = Z_local @ Wo[local_rows].
The host sums the two partials per batch and adds bo (no collective).

Per-core DRAM inputs:
  xt  : [D, S]   bf16  X[b]^T (full model dims, all tokens)
  wq/wk/wv : [D, DL]  bf16  column-slice for this head-half
  wo  : [DL, D]  bf16  row-slice for this head-half
  bq/bk/bv : [DL] fp32
Output:
  ot  : [D, S]  fp32  partial O^T for this batch (host: sum pair + ^T + bo)

On-chip layouts (P=128 partitions):
  xt_sb[p, c, t] = X^T[c*128+p, t]                  bf16
  kt[pr][p, t]   = K^T[pr*128+p, t]   (pr = local pair of heads)
  qt2[qb][p, pr, q] = Q^T[pr*128+p, qc*512+q]  (double-buffered per qc)
  vt[pr][p, tk, h*65+j] = V[tk*128+p, pr*128+h*64+j] for j<64,
                          1.0 for j==64 (ones col -> softmax denom)
  zt[pr][p, q]   = normalized Z^T

Attention inner loop per (qc, pr): 16 kc chunks; per kc two row-packed
score matmuls (contraction 64 per head, PE rows 0-63 / 64-127) into a
2-bank PSUM tile, one ACT exp -> bf16 probs, and two M=65 PV matmuls
accumulating za/zb (row 64 = denominator).  PV is emitted LAG=2 kc
behind scores so the PE stream never blocks on the scalar engine.
Q-proj for the next qc and O-proj for the previous qc are emitted as
filler chains inside the kc stream (separate 1-bank PSUM pool).
"""

import numpy as np
import ml_dtypes
from contextlib import ExitStack

import concourse.bass as bass
import concourse.tile as tile
from concourse import bacc, mybir, library_config

GPB = True   # normalize via gpsimd partition_broadcast (else DRAM bounce)

F32 = mybir.dt.float32
BF16 = mybir.dt.bfloat16
P = 128


def build_attention_nc(S=2048, D=1024, DL=512):
    NPAIR = DL // P           # 4 local head pairs
    DIN = D // P              # 8 input-dim chunks
    KC = S // P               # 16 key chunks
    QCN = S // 512            # 4 query chunks
    NTOK = S // P             # 16 token chunks (V proj)
    LAG = 2                   # PV lags scores by LAG kc steps

    nc = bacc.Bacc("TRN2", target_bir_lowering=False, debug=False)

    xt_d = nc.dram_tensor("xt", [D, S], BF16, kind="ExternalInput").ap()
    wq_d = nc.dram_tensor("wq", [D, DL], BF16, kind="ExternalInput").ap()
    wk_d = nc.dram_tensor("wk", [D, DL], BF16, kind="ExternalInput").ap()
    wv_d = nc.dram_tensor("wv", [D, DL], BF16, kind="ExternalInput").ap()
    wo_d = nc.dram_tensor("wo", [DL, D], BF16, kind="ExternalInput").ap()
    bq_d = nc.dram_tensor("bq", [DL], F32, kind="ExternalInput").ap()
    bk_d = nc.dram_tensor("bk", [DL], F32, kind="ExternalInput").ap()
    bv_d = nc.dram_tensor("bv", [DL], F32, kind="ExternalInput").ap()
    ot_d = nc.dram_tensor("ot", [D, S], F32, kind="ExternalOutput").ap()

    xt_r = xt_d.rearrange("(c p) t -> p c t", p=P)
    wq_r = wq_d.rearrange("(c p) n -> p c n", p=P)
    wk_r = wk_d.rearrange("(c p) n -> p c n", p=P)
    wv_r = wv_d.rearrange("(c p) n -> p c n", p=P)
    wo_r = wo_d.rearrange("(c p) n -> p c n", p=P)
    bq_r = bq_d.rearrange("(c p) -> p c", p=P)
    bk_r = bk_d.rearrange("(c p) -> p c", p=P)
    bv_r = bv_d.rearrange("(a d) -> a d", a=1)

    EXP = mybir.ActivationFunctionType.Exp

    with tile.TileContext(nc) as tc, ExitStack() as ctx:
        const = ctx.enter_context(tc.tile_pool(name="const", bufs=1))
        big = ctx.enter_context(tc.tile_pool(name="big", bufs=1))
        wpool = ctx.enter_context(tc.tile_pool(name="wpool", bufs=3))
        qpool = ctx.enter_context(tc.tile_pool(name="qpool", bufs=2))
        work = ctx.enter_context(tc.tile_pool(name="work", bufs=3))
        probs_pool = ctx.enter_context(tc.tile_pool(name="probs", bufs=5))
        # PSUM budget (8 banks): scores 2x2 + z 3 + proj-filler 1
        spsum = ctx.enter_context(tc.tile_pool(name="spsum", bufs=2, space="PSUM"))
        zpsum = ctx.enter_context(tc.tile_pool(name="zpsum", bufs=3, space="PSUM"))
        ppsum = ctx.enter_context(tc.tile_pool(name="ppsum", bufs=1, space="PSUM"))
        dramp = ctx.enter_context(tc.tile_pool(name="dramp", bufs=2, space="DRAM"))

        # ---- constants ----
        bq_t = const.tile([P, NPAIR], F32)
        nc.sync.dma_start(bq_t[:], bq_r[:, :])
        bk_t = const.tile([P, NPAIR], F32)
        nc.sync.dma_start(bk_t[:], bk_r[:, :])
        bvb = const.tile([P, DL], F32)
        nc.sync.dma_start(bvb[:], bv_r[0:1, :].to_broadcast((P, DL)))

        if GPB:
            nc.gpsimd.load_library(library_config.attn)

        # ---- X^T + Wk loads, pieced so the first K-proj chain starts
        # as soon as wk + the first 512 token columns have landed ----
        xt_t = big.tile([P, DIN, S], BF16, tag="xt")
        wk_t = wpool.tile([P, DIN, DL], BF16, tag="w", name="wk_t")
        for c in range(DIN):
            nc.sync.dma_start(wk_t[:, c, :], wk_r[:, c, :])
            nc.sync.dma_start(xt_t[:, c, 0:512], xt_r[:, c, 0:512])
        for t in range(1, QCN):
            for c in range(DIN):
                nc.sync.dma_start(xt_t[:, c, t * 512:(t + 1) * 512],
                                  xt_r[:, c, t * 512:(t + 1) * 512])

        kt = [big.tile([P, S], BF16, tag=f"kt{p}", name=f"kt{p}")
              for p in range(NPAIR)]
        vt = [big.tile([P, NTOK, 130], BF16, tag=f"vt{p}", name=f"vt{p}")
              for p in range(NPAIR)]
        zt = [big.tile([P, S], BF16, tag=f"zt{p}", name=f"zt{p}")
              for p in range(NPAIR)]

        # ones columns of vt (col 64 of each head's 65-wide block)
        for pr in range(NPAIR):
            ones_ap = vt[pr].rearrange("p t (h c) -> p t h c", c=65)[:, :, :, 64:65]
            nc.vector.memset(ones_ap, 1.0)

        def load_w(w_r, nm, depth=DIN, width=DL):
            w_t = wpool.tile([P, depth, width], BF16, tag="w", name=nm)
            for c in range(depth):
                nc.sync.dma_start(w_t[:, c, :], w_r[:, c, :])
            return w_t

        wv_t = load_w(wv_r, "wv_t")

        # ---- K^T projection, all pairs (startup: uses spsum dbuf);
        # t-outer so the first chains only need the first xt pieces ----
        for t in range(QCN):
            for pr in range(NPAIR):
                ps = spsum.tile([P, 2, 512], F32, tag="sc", name="psk")
                for c in range(DIN):
                    nc.tensor.matmul(
                        ps[:, 0, :],
                        wk_t[:, c, pr * P:(pr + 1) * P],
                        xt_t[:, c, t * 512:(t + 1) * 512],
                        start=(c == 0), stop=(c == DIN - 1),
                    )
                nc.vector.tensor_scalar_add(
                    kt[pr][:, t * 512:(t + 1) * 512], ps[:, 0, :],
                    bk_t[:, pr:pr + 1])

        wq_t = load_w(wq_r, "wq_t")

        # ---- V projection, all token chunks ----
        for t in range(NTOK):
            ps = spsum.tile([P, 2, 512], F32, tag="sc", name="psv")
            for c in range(DIN):
                nc.tensor.matmul(
                    ps[:, 0, :],
                    xt_t[:, c, t * P:(t + 1) * P],
                    wv_t[:, c, :],
                    start=(c == 0), stop=(c == DIN - 1),
                )
            for pr in range(NPAIR):
                dst = vt[pr].rearrange(
                    "p t (h c) -> p t h c", c=65)[:, t, :, 0:64]
                src = ps[:, 0, pr * P:(pr + 1) * P].rearrange(
                    "p (h c) -> p h c", c=64)
                bsrc = bvb[:, pr * P:(pr + 1) * P].rearrange(
                    "p (h c) -> p h c", c=64)
                nc.vector.tensor_add(dst, src, bsrc)

        # ---- Q^T projection helper (qc granularity, double-buffered) ----
        def qproj(qc, pool_):
            qt = qpool.tile([P, NPAIR, 512], BF16, tag="qt", name=f"qt{qc % 2}")
            for pr in range(NPAIR):
                ps = pool_.tile([P, 2, 512] if pool_ is spsum else [P, 512],
                                F32, tag="sc" if pool_ is spsum else "proj",
                                name="psq")
                pview = ps[:, 0, :] if pool_ is spsum else ps[:]
                for c in range(DIN):
                    nc.tensor.matmul(
                        pview,
                        wq_t[:, c, pr * P:(pr + 1) * P],
                        xt_t[:, c, qc * 512:(qc + 1) * 512],
                        start=(c == 0), stop=(c == DIN - 1),
                    )
                nc.vector.tensor_scalar_add(
                    qt[:, pr, :], pview, bq_t[:, pr:pr + 1])
            return qt

        qt_cur = qproj(0, spsum)
        wo_t = load_w(wo_r, "wo_t", depth=NPAIR, width=D)

        # ---- filler chain generators (run inside the kc stream) ----
        def oproj_chain(qc, ec, tail=False):
            if tail and ec % 2 == 0:
                # attention is over: alternate with the scores pool so the
                # DVE evictions double-buffer instead of serializing
                pst = spsum.tile([P, 2, 512], F32, tag="sc", name="psot")
                ps = pst[:, 0, :]
            else:
                ps = ppsum.tile([P, 512], F32, tag="proj", name="pso")[:]
            for dc in range(NPAIR):
                nc.tensor.matmul(
                    ps,
                    wo_t[:, dc, ec * P:(ec + 1) * P],
                    zt[dc][:, qc * 512:(qc + 1) * 512],
                    start=(dc == 0), stop=(dc == NPAIR - 1),
                )
            st = work.tile([P, 512], F32, tag="stage", name="st", bufs=2)
            nc.vector.tensor_copy(st[:], ps)
            nc.sync.dma_start(
                ot_d[ec * P:(ec + 1) * P, qc * 512:(qc + 1) * 512], st[:])

        def qproj_chain(qc, pr, qt):
            ps = ppsum.tile([P, 512], F32, tag="proj", name="psq")
            for c in range(DIN):
                nc.tensor.matmul(
                    ps[:],
                    wq_t[:, c, pr * P:(pr + 1) * P],
                    xt_t[:, c, qc * 512:(qc + 1) * 512],
                    start=(c == 0), stop=(c == DIN - 1),
                )
            nc.vector.tensor_scalar_add(qt[:, pr, :], ps[:], bq_t[:, pr:pr + 1])

        # ---- normalize + evict Z^T for one (pr, qc) ----
        # za/zb are staged to SBUF immediately (split across VectorE and
        # ScalarE) so their PSUM banks free up before the next pair's PV
        # needs them; the DMA-bounce broadcast and the normalize muls then
        # run entirely SBUF-side off the PE critical path.
        def normalize(pr, qc, za, zb):
            qsl = slice(qc * 512, (qc + 1) * 512)
            zsa = work.tile([P, 2, 512], F32, tag="zsa", name="zsa", bufs=2)
            nc.vector.tensor_copy(zsa[0:65, 0, :], za[0:65, :])
            nc.scalar.copy(zsa[0:65, 1, :], zb[0:65, :])
            rb = work.tile([P, 2, 512], F32, tag="rbc", name="rb", bufs=2)
            if GPB:
                # shift denom rows to partition 0 (SBUF->SBUF DMA), take
                # reciprocal there, then gpsimd-broadcast to 64 partitions
                rsh = work.tile([P, 2, 512], F32, tag="rsh", name="rsh",
                                bufs=2)
                nc.sync.dma_start(rsh[0:1, :, :], zsa[64:65, :, :])
                rcp = work.tile([P, 2, 512], F32, tag="rcp", name="rcp",
                                bufs=2)
                nc.vector.reciprocal_approx_fast(rcp[0:1, :, :],
                                                 rsh[0:1, :, :])
                nc.gpsimd.partition_broadcast(rb[0:64, :, :],
                                              rcp[0:1, :, :], channels=64)
            else:
                rsc = dramp.tile([2, 512], F32, tag="rsc", name="rsc")
                nc.sync.dma_start(rsc[0:1, :], zsa[64:65, 0, :])
                nc.sync.dma_start(rsc[1:2, :], zsa[64:65, 1, :])
                rbr = work.tile([P, 2, 512], F32, tag="rbc", name="rbr")
                nc.sync.dma_start(rbr[0:64, 0, :],
                                  rsc[0:1, :].to_broadcast((64, 512)))
                nc.sync.dma_start(rbr[0:64, 1, :],
                                  rsc[1:2, :].to_broadcast((64, 512)))
                nc.vector.reciprocal_approx_fast(rb[0:64, :, :],
                                                 rbr[0:64, :, :])
            nc.vector.tensor_mul(zt[pr][0:64, qsl], zsa[0:64, 0, :],
                                 rb[0:64, 0, :])
            zs = work.tile([P, 512], BF16, tag="zstage", name="zs")
            nc.vector.tensor_mul(zs[0:64, :], zsa[0:64, 1, :],
                                 rb[0:64, 1, :])
            nc.sync.dma_start(zt[pr][64:128, qsl], zs[0:64, :])

        # ---- attention steady loop ----
        pv_q = []          # pending PV work: (pq, pr, qc, kc, za, zb)
        zacc = {}          # (pr, qc) -> (za, zb)
        fillers = []       # callables emitting one PE chain each

        def emit_pv(item):
            pq, pr, qc, kc, za, zb = item
            vpr = vt[pr].rearrange("p t (h c) -> p t h c", c=65)
            nc.tensor.matmul(
                za[0:65, :], vpr[:, kc, 0, :], pq[:, 0, :],
                start=(kc == 0), stop=(kc == KC - 1),
            )
            nc.tensor.matmul(
                zb[0:65, :], vpr[:, kc, 1, :], pq[:, 1, :],
                start=(kc == 0), stop=(kc == KC - 1),
            )
            if kc == KC - 1:
                normalize(pr, qc, za, zb)

        FILL_SLOTS = (3, 7, 11)   # kc positions where one filler chain runs

        for qc in range(QCN):
            qt_use = qt_cur
            # schedule fillers for this qc: Q-proj(qc+1) during pr3,
            # O-proj(qc-1) spread over pr0..pr2
            qnext = [None]
            if qc + 1 < QCN:
                qt_next = qpool.tile([P, NPAIR, 512], BF16, tag="qt",
                                     name=f"qt{(qc + 1) % 2}")
                qnext[0] = qt_next
            for pr in range(NPAIR):
                qsl = slice(qc * 512, (qc + 1) * 512)
                za = zpsum.tile([P, 512], F32, tag="z", name=f"za{pr}_{qc}")
                zb = zpsum.tile([P, 512], F32, tag="z", name=f"zb{pr}_{qc}")
                zacc[(pr, qc)] = (za, zb)
                for kc in range(KC):
                    sq = spsum.tile([P, 2, 512], F32, tag="sc", name="sq")
                    nc.tensor.matmul(
                        sq[:, 0, :],
                        kt[pr][0:64, kc * P:(kc + 1) * P],
                        qt_use[0:64, pr, :],
                        start=True, stop=True,
                    )
                    nc.tensor.matmul(
                        sq[:, 1, :],
                        kt[pr][64:128, kc * P:(kc + 1) * P],
                        qt_use[64:128, pr, :],
                        start=True, stop=True,
                    )
                    pq = probs_pool.tile([P, 2, 512], BF16, tag="pq",
                                         name="pq")
                    nc.scalar.activation(pq[:], sq[:], EXP, scale=0.125)
                    pv_q.append((pq, pr, qc, kc, za, zb))
                    if len(pv_q) > LAG:
                        emit_pv(pv_q.pop(0))
                    if kc in FILL_SLOTS and fillers:
                        fillers.pop(0)()
                # queue fillers now that this pair's scores are done
                if pr == 2 and qnext[0] is not None:
                    qt_next = qnext[0]
                    for fpr in range(NPAIR):
                        fillers.append(
                            lambda q=qc + 1, p=fpr, t=qt_next:
                            qproj_chain(q, p, t))
                if pr == 0 and qc > 0:
                    for ec in range(DIN):
                        fillers.append(
                            lambda q=qc - 1, e=ec: oproj_chain(q, e))
            if qnext[0] is not None:
                qt_cur = qnext[0]

        # drain PV queue, remaining fillers, and final O-proj
        while pv_q:
            emit_pv(pv_q.pop(0))
        while fillers:
            fillers.pop(0)()
        for ec in range(DIN):
            oproj_chain(QCN - 1, ec, tail=True)

    nc.compile()
    return nc


# ---------------- host-side entry point ----------------

BF = ml_dtypes.bfloat16
_B, _S, _D, _H = 4, 2048, 1024, 16
_DL = _D // 2
_NC_CACHE = None


def _get_nc():
    global _NC_CACHE
    if _NC_CACHE is None:
        _NC_CACHE = build_attention_nc(S=_S, D=_D, DL=_DL)
    return _NC_CACHE


def kernel(X, Wq, bq, Wk, bk, Wv, bv, Wo, bo):
    """Full-input multi-head attention on 8 TRN2 NeuronCores.

    Sharding: core c handles batch c//2 and head-half c%2 (8 heads).
    Each core returns a partial O^T (its heads' contribution); the host
    sums the two partials per batch and adds bo. No collectives.
    """
    from concourse.bass_utils import run_bass_kernel_spmd

    X = np.asarray(X, dtype=np.float32)
    bo = np.asarray(bo, dtype=np.float32)
    wq_f = np.asarray(Wq, dtype=np.float32)
    wk_f = np.asarray(Wk, dtype=np.float32)
    wv_f = np.asarray(Wv, dtype=np.float32)
    wo_f = np.asarray(Wo, dtype=np.float32)
    bq_f = np.asarray(bq, dtype=np.float32)
    bk_f = np.asarray(bk, dtype=np.float32)
    bv_f = np.asarray(bv, dtype=np.float32)

    xts = [np.ascontiguousarray(X[b].T.astype(BF)) for b in range(_B)]
    in_maps = []
    for c in range(8):
        b, hh = c // 2, c % 2
        dsl = slice(hh * _DL, (hh + 1) * _DL)
        in_maps.append({
            "xt": xts[b],
            "wq": np.ascontiguousarray(wq_f[:, dsl].astype(BF)),
            "wk": np.ascontiguousarray(wk_f[:, dsl].astype(BF)),
            "wv": np.ascontiguousarray(wv_f[:, dsl].astype(BF)),
            "wo": np.ascontiguousarray(wo_f[dsl, :].astype(BF)),
            "bq": np.ascontiguousarray(bq_f[dsl]),
            "bk": np.ascontiguousarray(bk_f[dsl]),
            "bv": np.ascontiguousarray(bv_f[dsl]),
        })

    nc = _get_nc()
    res = run_bass_kernel_spmd(nc, in_maps, list(range(8)))

    out = np.empty((_B, _S, _D), np.float32)
    for b in range(_B):
        acc = res.results[2 * b]["ot"] + res.results[2 * b + 1]["ot"]
        out[b] = acc.T + bo[None, :]
    return out
